# revision 15
# baseline (speedup 1.0000x reference)
"""Multi-head attention (B=4, N=2048, C=1024, H=8, Dh=128) on 8 TRN2 NeuronCores.

Sharding: head-split tensor parallel. Core c handles batch c//2 and heads
4*(c%2)..4*(c%2)+3, all 2048 queries. No device collectives: K/Q/V are
projected only for the core's own 4 heads; each core emits a partial output
projection (with half the effective output bias) and the host sums the two
partials per batch. SPMD: all cores run one graph, per-core weight slices.

v4: fully software-pipelined single phase. ft-major Q/K projection so the
attention/exp stream for head 0 starts ~20us in; all remaining matmul work
(Q/K proj heads 1-3, deferred V projection, output projection) rides as
filler "jobs" woven into the attention groups: each job sits in a pre- or
mid-slot position separated by >=1.7us of scores/PV matmuls, so the single
job psum bank (pq) never stalls the PE on its own drain. Pre-loop
projection chains borrow the (still idle) 3-deep score-psum ring. DMA
issue bandwidth is the scarce startup resource (~1us/op per DGE
processor): x rides as four 1MB strided DMAs (2KB descriptors) split
gpsimd(ph0)/sync(ph1), weights are host-pre-transposed contiguous rows on
sync, w0 tiles on the (otherwise idle) scalar queue, and the scalar queue
carries no early DMAs that could head-block the exp stream. Score psum
keeps the 3-deep ring (6 banks) so ACT streams exps back-to-back; jobs 1
bank, PV 1 bank. Q/K/V drains on DVE; tail outproj drains on the by-then
idle ACT; y writes alternate gpsimd/scalar. The last group's XBAR
transpose is split in two so the final PV isn't stuck behind a 10us op.

Math per core (fp16 matmuls, fp32 psum): scores = Q K^T (scale folded
into Wq; K-bias dropped, V-bias folded into b0 on host), softmax =
exp(s - sampledmax - 66) via one ACT pass per half (per-partition bias
AP + accum_out rowsum). Probs are normalized IN [q, k] LAYOUT, then
transposed to [key, query] tiles by the DMA XBAR transpose on the sync
HWDGE queue. Groups are software-pipelined: group g emits scores/exp
with the PV matmuls of group g-2 interleaved 4-per-query-tile. Output
y.T [1024 cout, 2048 tok] fp16 partial.
"""

import sys

if "/opt/trn_rl_repo" not in sys.path:
    sys.path.insert(0, "/opt/trn_rl_repo")

from contextlib import ExitStack

import numpy as np

import concourse.bass as bass
import concourse.mybir as mybir
from concourse import bacc
from concourse.bass_utils import run_bass_kernel_spmd
from concourse.tile import TileContext

F32 = mybir.dt.float32
BF16 = mybir.dt.bfloat16
FP16 = mybir.dt.float16
AF = mybir.ActivationFunctionType
ALU = mybir.AluOpType

DIM = 1024
HEADS = 8
HD = 128  # head dim
B, N = 4, 2048
SCALE = float(np.sqrt(DIM / HEADS))
NCORES = 8
TOK = 2048          # query tokens per core (whole batch)
KEYS = 2048         # keys per core (whole batch)
MARGIN = 66.0       # exp bias safety margin below sampled max


def _build():
    nc = bacc.Bacc("TRN2", target_bir_lowering=False, debug=False, num_devices=NCORES)

    # all weight layouts are already [partition, cin-chunk, free] on the host
    xT_e = nc.declare_dram_parameter("xT", [2, 8, 128, 1024], FP16, isOutput=False)
    wqT_e = nc.declare_dram_parameter("wqT", [4, 128, 8, 128], FP16, isOutput=False)
    wkT_e = nc.declare_dram_parameter("wkT", [4, 128, 8, 128], FP16, isOutput=False)
    wvT_e = nc.declare_dram_parameter("wvT", [128, 8, 512], FP16, isOutput=False)
    w0T_e = nc.declare_dram_parameter("w0T", [8, 128, 4, 128], FP16, isOutput=False)
    bq_e = nc.declare_dram_parameter("bq", [128, 4], F32, isOutput=False)
    b0_e = nc.declare_dram_parameter("b0", [128, 8], F32, isOutput=False)
    out_e = nc.declare_dram_parameter("out", [DIM, TOK], FP16, isOutput=True)
    HL = 4  # local heads per core

    with TileContext(nc) as tc, ExitStack() as ctx:
        persist = ctx.enter_context(tc.tile_pool(name="persist", bufs=1))
        QT = persist.tile([128, 4, TOK], FP16)         # [d, lhead, qtok]
        KT = persist.tile([128, 4, KEYS], FP16)        # [d, lhead, key]
        V = persist.tile([128, 16, 512], BF16)         # [tok%128, keytile, lfeat]
        bq_s = persist.tile([128, 4], F32)
        b0_s = persist.tile([128, 8], F32)
        wv0 = persist.tile([128, 8, 512], FP16)
        wqs = [persist.tile([128, 8, 128], FP16, name=f"wq{ft}", tag=f"wq{ft}")
               for ft in range(4)]
        wks = [persist.tile([128, 8, 128], FP16, name=f"wk{ft}", tag=f"wk{ft}")
               for ft in range(4)]
        w0s = [persist.tile([128, 4, 128], FP16, name=f"w0{ct}", tag=f"w0{ct}")
               for ct in range(8)]
        xpool = ctx.enter_context(tc.tile_pool(name="xT", bufs=1))
        xts = [xpool.tile([128, 8, 1024], FP16, name=f"xt{ph}", tag=f"xt{ph}")
               for ph in range(2)]

        # ---- DMA issue plan (issue cost ~1us/op; HWDGE shared sync+scalar,
        # SWDGE separate). gpsimd: x ph0. sync: first-use-ordered weights +
        # x ph1. scalar: w0 tiles only (exp stream starts ~20us).
        nc.gpsimd.dma_start(
            out=xts[0][:, 0:4, :], in_=xT_e[0, 0:4].rearrange("c p f -> p c f"))
        nc.gpsimd.dma_start(
            out=xts[0][:, 4:8, :], in_=xT_e[0, 4:8].rearrange("c p f -> p c f"))
        nc.sync.dma_start(out=bq_s[:, :], in_=bq_e[:, :])
        nc.sync.dma_start(out=wqs[0][:, :, :], in_=wqT_e[0])
        nc.sync.dma_start(out=wks[0][:, :, :], in_=wkT_e[0])
        nc.sync.dma_start(
            out=xts[1][:, 0:4, :], in_=xT_e[1, 0:4].rearrange("c p f -> p c f"))
        nc.sync.dma_start(
            out=xts[1][:, 4:8, :], in_=xT_e[1, 4:8].rearrange("c p f -> p c f"))
        nc.sync.dma_start(out=wv0[:, :, :], in_=wvT_e[:, :, :])
        nc.sync.dma_start(out=b0_s[:, :], in_=b0_e[:, :])
        nc.sync.dma_start(out=wqs[1][:, :, :], in_=wqT_e[1])
        nc.sync.dma_start(out=wks[1][:, :, :], in_=wkT_e[1])
        for ft in (2, 3):
            nc.sync.dma_start(out=wqs[ft][:, :, :], in_=wqT_e[ft])
            nc.sync.dma_start(out=wks[ft][:, :, :], in_=wkT_e[ft])
        # w0 tiles ride late on sync (first use ~240us); the scalar queue
        # stays empty so the first exp isn't head-blocked behind DMAs
        for ct in range(8):
            nc.sync.dma_start(out=w0s[ct][:, :, :], in_=w0T_e[ct])

        # psum: scores 2x[128,1024] (4 banks) + jobs 2x[128,512] + PV
        # 2x[128,512] = 8 banks exactly. The 2-deep score ring makes the
        # exp stream latency-chained (~3.8us/qi), which keeps tensor-engine
        # duty moderate — the DVFS governor then holds boost clocks; denser
        # schedules measured SLOWER overall (chip drops to ~83% clock).
        spool = ctx.enter_context(tc.tile_pool(name="sc", bufs=2, space="PSUM"))
        pq = ctx.enter_context(tc.tile_pool(name="pq", bufs=2, space="PSUM"))
        opool = ctx.enter_context(tc.tile_pool(name="ov", bufs=2, space="PSUM"))

        upool = ctx.enter_context(tc.tile_pool(name="u", bufs=2))
        utpool = ctx.enter_context(tc.tile_pool(name="ut", bufs=2))
        otpool = ctx.enter_context(tc.tile_pool(name="ot", bufs=1))
        ypool = ctx.enter_context(tc.tile_pool(name="y", bufs=4))
        small = ctx.enter_context(tc.tile_pool(name="sm", bufs=16))
        OT_t = [otpool.tile([128, 4, 512], FP16, name=f"ott{i}", tag=f"ott{i}")
                for i in range(4)]

        # ---------------- PE job closures ----------------
        def qproj(ft, ph, tch, pool=None):
            def run():
                p = pool if pool is not None else pq
                ps = p.tile([128, 512], F32, name="qps",
                            tag="sc" if p is spool else "pq")
                for c in range(8):
                    nc.tensor.matmul(
                        ps[:, :], wqs[ft][:, c, :],
                        xts[ph][:, c, tch * 512:(tch + 1) * 512],
                        start=(c == 0), stop=(c == 7))
                nc.vector.tensor_scalar(
                    QT[:, ft, ph * 1024 + tch * 512:ph * 1024 + (tch + 1) * 512],
                    ps[:, :], bq_s[:, ft:ft + 1], None, op0=ALU.add)
            return run

        def kproj(ft, ph, tch, pool=None):
            def run():
                p = pool if pool is not None else pq
                ps = p.tile([128, 512], F32, name="kps",
                            tag="sc" if p is spool else "pq")
                for c in range(8):
                    nc.tensor.matmul(
                        ps[:, :], wks[ft][:, c, :],
                        xts[ph][:, c, tch * 512:(tch + 1) * 512],
                        start=(c == 0), stop=(c == 7))
                nc.vector.tensor_copy(
                    KT[:, ft, ph * 1024 + tch * 512:ph * 1024 + (tch + 1) * 512],
                    ps[:, :])
            return run

        def vproj(ph, tt):
            def run():
                ps = pq.tile([128, 512], F32, name="vps", tag="pq")
                for c in range(8):
                    nc.tensor.matmul(
                        ps[:, :], xts[ph][:, c, tt * 128:(tt + 1) * 128],
                        wv0[:, c, :], start=(c == 0), stop=(c == 7))
                nc.vector.tensor_copy(V[:, ph * 8 + tt, :], ps[:, :])
            return run

        def outjob(tch, ct, tail=False):
            def run():
                # tail chunks borrow the (by then idle) score-psum ring and
                # drain on the (by then idle) ACT engine
                if tail and ct % 2 == 0:
                    pool, tag = spool, "sc"
                else:
                    pool, tag = pq, "pq"
                ps = pool.tile([128, 512], F32, name="yps", tag=tag)
                for dc in range(4):
                    nc.tensor.matmul(
                        ps[:, :], w0s[ct][:, dc, :], OT_t[tch][:, dc, :],
                        start=(dc == 0), stop=(dc == 3))
                y = ypool.tile([128, 512], FP16, name="y", tag="y")
                if tail and ct % 2 == 0:
                    nc.scalar.activation(y[:, :], ps[:, :], AF.Identity,
                                         bias=b0_s[:, ct:ct + 1])
                else:
                    nc.vector.tensor_scalar(
                        y[:, :], ps[:, :], b0_s[:, ct:ct + 1], None,
                        op0=ALU.add)
                nc.gpsimd.dma_start(
                    out=out_e[ct * 128:(ct + 1) * 128,
                              tch * 512:(tch + 1) * 512],
                    in_=y[:, :])
            return run

        # -------- pre-attention: head-0 Q (ph0) + K chains, alternating
        # the two psum rings so consecutive chains pipeline
        qproj(0, 0, 0, spool)()
        qproj(0, 0, 1)()
        kproj(0, 0, 0, spool)()
        kproj(0, 0, 1)()
        # group-0 pass 1 (half-0 scores + exps), emitted BEFORE the K-ph1
        # and V chains so its mx/exp ops precede their drains in the DVE/ACT
        # static orders — the exp stream starts ~14us instead of ~39us
        UT40 = utpool.tile([128, 2, 4, 8, 128], BF16, name="UT4", tag="ut")
        ug0 = upool.tile([128, 2, 4, 1024], BF16, name="ug", tag="u")
        g0st = []
        for qi in range(4):
            q_sl = QT[:, 0, qi * 128:(qi + 1) * 128]
            negb = small.tile([128, 1], F32, tag="negb", name="negb")
            ra = small.tile([128, 1], F32, tag="ra", name="ra")
            ps = spool.tile([128, 1024], F32, tag="sc", name="ps")
            for kc in range(2):
                nc.tensor.matmul(
                    ps[:, kc * 512:(kc + 1) * 512], q_sl,
                    KT[:, 0, kc * 512:(kc + 1) * 512],
                    start=True, stop=True)
            mx = small.tile([128, 1], F32, tag="mx", name="mx")
            with tc.high_priority(offset=30):
                nc.vector.tensor_reduce(
                    mx[:, :],
                    ps[:, :].rearrange("p (n s) -> p n s", s=4)[:, :, 0],
                    axis=mybir.AxisListType.X, op=ALU.max)
            nc.vector.tensor_scalar(
                negb[:, :], mx[:, :], -1.0, -MARGIN,
                op0=ALU.mult, op1=ALU.add)
            nc.scalar.activation(
                ug0[:, 0, qi, :], ps[:, :], AF.Exp,
                bias=negb[:, :], scale=1.0, accum_out=ra[:, :])
            g0st.append((q_sl, negb, ra))
        kproj(0, 1, 0, spool)()
        kproj(0, 1, 1)()
        vproj(0, 0)()
        vproj(0, 1)()
        vproj(0, 2)()
        vproj(0, 3)()

        # ---------------- job schedule ----------------
        # pre[(gi,qi)] emitted before the slot's scores, mid[(gi,qi)] between
        # the scores pair and the PV chunk — so consecutive jobs are always
        # separated by >=1.7us of other matmuls (single pq bank, no stalls).
        # V tile kt=ph*8+tt before PV chunk kt//4 of g2; head h's proj before
        # group 4h; outproj(0) in g15 (OT0 completes at the end of g14).
        jobs = {
            (0, 0): [qproj(0, 1, 0), vproj(0, 4)],
            (0, 1): [vproj(0, 5)],
            (0, 2): [vproj(0, 6)],
            (0, 3): [vproj(0, 7)],
            (1, 0): [qproj(0, 1, 1), vproj(1, 0)],
            (1, 1): [vproj(1, 1)],
            (1, 2): [vproj(1, 2)],
            (1, 3): [vproj(1, 3)],
            (2, 0): [vproj(1, 4)],
            (2, 1): [vproj(1, 5)],
            (2, 2): [qproj(1, 0, 0), vproj(1, 6)],
            (2, 3): [vproj(1, 7)],
            (3, 0): [kproj(1, 0, 0)],
            (3, 1): [qproj(1, 0, 1), kproj(1, 0, 1)],
            (3, 2): [qproj(1, 1, 0), kproj(1, 1, 0)],
            (3, 3): [qproj(1, 1, 1), kproj(1, 1, 1)],
            (4, 1): [qproj(2, 0, 0)],
            (4, 3): [qproj(2, 0, 1)],
            (5, 1): [qproj(2, 1, 0)],
            (5, 3): [qproj(2, 1, 1)],
            (6, 1): [kproj(2, 0, 0)],
            (6, 3): [kproj(2, 0, 1)],
            (7, 0): [kproj(2, 1, 0)],
            (7, 2): [kproj(2, 1, 1)],
            (8, 1): [qproj(3, 0, 0)],
            (8, 3): [qproj(3, 0, 1)],
            (9, 1): [qproj(3, 1, 0)],
            (9, 3): [qproj(3, 1, 1)],
            (10, 1): [kproj(3, 0, 0)],
            (10, 3): [kproj(3, 0, 1)],
            (11, 0): [kproj(3, 1, 0)],
            (11, 2): [kproj(3, 1, 1)],
        }
        for qi in range(4):
            jobs[(15, qi)] = [outjob(0, 2 * qi), outjob(0, 2 * qi + 1)]

        # ---------------- attention ----------------
        pends = []  # (h, qg, UT4)

        def pv_chunk(pend, ps_o, qi):
            h, qg, UT4 = pend
            for kt in range(qi * 4, qi * 4 + 4):
                nc.tensor.matmul(
                    ps_o[:, :], V[:, kt, h * 128:(h + 1) * 128],
                    UT4[:, kt // 8, :, kt % 8, :],
                    start=(kt == 0), stop=(kt == 15))

        def pv_tail(pend, ps_o):
            h, qg, UT4 = pend
            nc.vector.tensor_copy(OT_t[qg][:, h, :], ps_o[:, :])

        def flush_pv(pend):
            ps_o = opool.tile([128, 512], F32, tag="ov", name="ps_o")
            for qi in range(4):
                pv_chunk(pend, ps_o, qi)
            pv_tail(pend, ps_o)

        gi = 0
        for h in range(HL):
            for qg in range(4):
                # [k%128, qi, keytile, q%128]: fully contiguous — the XBAR
                # transpose requires a contiguous destination
                # [k%128, keyhalf, qi, keytile%8, q%128]: PV chunks 0-1
                # contract keys 0-1023 = the FIRST transpose half, so the PV
                # critical path waits only a 1MB XBAR, not the full 2MB
                if gi == 0:
                    UT4, ug = UT40, ug0
                else:
                    UT4 = utpool.tile([128, 2, 4, 8, 128], BF16, name="UT4",
                                      tag="ut")
                    ug = upool.tile([128, 2, 4, 1024], BF16, name="ug",
                                    tag="u")
                ready = pends.pop(0) if len(pends) >= 2 else None
                ps_o = None
                if ready is not None:
                    ps_o = opool.tile([128, 512], F32, tag="ov", name="ps_o")
                if gi == 0:
                    # pass 1 ran in the pre-loop; here: jobs + half-1 + norms
                    for qi in range(4):
                        for job in jobs.get((gi, qi), ()):
                            job()
                        q_sl, negb, ra = g0st[qi]
                        rb = small.tile([128, 1], F32, tag="rb", name="rb")
                        ps = spool.tile([128, 1024], F32, tag="sc", name="ps")
                        for kc in range(2):
                            nc.tensor.matmul(
                                ps[:, kc * 512:(kc + 1) * 512], q_sl,
                                KT[:, 0, 1024 + kc * 512:
                                   1024 + (kc + 1) * 512],
                                start=True, stop=True)
                        nc.scalar.activation(
                            ug[:, 1, qi, :], ps[:, :], AF.Exp,
                            bias=negb[:, :], scale=1.0, accum_out=rb[:, :])
                        nc.vector.tensor_tensor(
                            out=ra[:, :], in0=ra[:, :], in1=rb[:, :],
                            op=ALU.add)
                        rc1 = small.tile([128, 1], F32, tag="rc1", name="rc1")
                        nc.vector.reciprocal(rc1[:, :], ra[:, :])
                        nc.vector.tensor_scalar(
                            ug[:, :, qi, :], ug[:, :, qi, :], rc1[:, :], None,
                            op0=ALU.mult)
                    nc.sync.dma_start(out=UT4[:, 0], in_=ug[:, 0],
                                      transpose=True)
                    nc.sync.dma_start(out=UT4[:, 1], in_=ug[:, 1],
                                      transpose=True)
                    gi += 1
                    pends.append((h, qg, UT4))
                    continue
                for qi in range(4):
                    for job in jobs.get((gi, qi), ()):
                        job()
                    qt = qg * 4 + qi
                    q_sl = QT[:, h, qt * 128:(qt + 1) * 128]

                    negb = small.tile([128, 1], F32, tag="negb", name="negb")
                    ra = small.tile([128, 1], F32, tag="ra", name="ra")
                    rb = small.tile([128, 1], F32, tag="rb", name="rb")
                    ps_halves = []
                    for half in range(2):
                        ps = spool.tile([128, 1024], F32, tag="sc", name="ps")
                        ps_halves.append(ps)
                        for kc in range(2):
                            nc.tensor.matmul(
                                ps[:, kc * 512:(kc + 1) * 512], q_sl,
                                KT[:, h, half * 1024 + kc * 512:
                                   half * 1024 + (kc + 1) * 512],
                                start=True, stop=True)
                    if ready is not None:
                        pv_chunk(ready, ps_o, qi)
                    mx = small.tile([128, 1], F32, tag="mx", name="mx")
                    with tc.high_priority(offset=30):
                        nc.vector.tensor_reduce(
                            mx[:, :],
                            ps_halves[0][:, :].rearrange(
                                "p (n s) -> p n s", s=4)[:, :, 0],
                            axis=mybir.AxisListType.X, op=ALU.max)
                    # on DVE right after the mx reduce — avoids a
                    # cross-engine hop on the scores->exp latency chain
                    nc.vector.tensor_scalar(
                        negb[:, :], mx[:, :], -1.0, -MARGIN,
                        op0=ALU.mult, op1=ALU.add)
                    for half in range(2):
                        nc.scalar.activation(
                            ug[:, half, qi, :], ps_halves[half][:, :],
                            AF.Exp, bias=negb[:, :], scale=1.0,
                            accum_out=(ra if half == 0 else rb)[:, :])

                    # normalize the probs in [q, k] layout, where the
                    # reciprocal rowsum is a per-partition scalar — PV
                    # output needs no further normalization
                    nc.vector.tensor_tensor(
                        out=ra[:, :], in0=ra[:, :], in1=rb[:, :],
                        op=ALU.add)
                    rc1 = small.tile([128, 1], F32, tag="rc1", name="rc1")
                    nc.vector.reciprocal(rc1[:, :], ra[:, :])
                    nc.vector.tensor_scalar(
                        ug[:, :, qi, :], ug[:, :, qi, :], rc1[:, :], None,
                        op0=ALU.mult)

                if ready is not None:
                    pv_tail(ready, ps_o)
                # two XBAR transposes per group (one per key half) on the
                # sync HWDGE queue: the first 1MB half unblocks PV chunks 0-1
                nc.sync.dma_start(out=UT4[:, 0], in_=ug[:, 0], transpose=True)
                nc.sync.dma_start(out=UT4[:, 1], in_=ug[:, 1], transpose=True)
                gi += 1
                pends.append((h, qg, UT4))

        # ---------------- tail ----------------
        # outproj(0) ran as g15 jobs. OT1 completed during g15 (pv_tail of
        # g13). flush(g14) -> OT2; outproj(2) covers the g15 XBAR wait;
        # flush(g15) -> OT3.
        flush_pv(pends.pop(0))
        for ct in range(8):
            outjob(1, ct, tail=True)()
        for ct in range(8):
            outjob(2, ct, tail=True)()
        flush_pv(pends.pop(0))
        for ct in range(8):
            outjob(3, ct, tail=True)()

    nc.compile()
    return nc


_NC = None


def _get_nc():
    global _NC
    if _NC is None:
        _NC = _build()
    return _NC


def _make_in_maps(x, W_qkv, b_qkv, W0, b0):
    x = np.asarray(x, dtype=np.float32)
    W_qkv = np.asarray(W_qkv, dtype=np.float32)
    b_qkv = np.asarray(b_qkv, dtype=np.float32)
    W0 = np.asarray(W0, dtype=np.float32)
    b0 = np.asarray(b0, dtype=np.float32)

    def tile_w(wT, fsz):
        # [1024 cin, F] -> [F/fsz, 128, 8, fsz] contiguous ([p, c, f] rows)
        nf = wT.shape[1] // fsz
        return np.ascontiguousarray(
            wT.reshape(8, 128, nf, fsz).transpose(2, 1, 0, 3)
        ).astype(np.float16)

    # V-bias folds through the output projection (softmax rows sum to 1);
    # K-bias only shifts each score row uniformly, which softmax cancels.
    # Each core of a pair adds half of the effective output bias.
    b0_eff = 0.5 * (b0 + W0 @ b_qkv[2 * DIM:3 * DIM])
    b0r = np.ascontiguousarray(b0_eff.reshape(8, 128).T).astype(np.float32)

    in_maps = []
    for c in range(NCORES):
        b, g = c // 2, c % 2
        hs = slice(g * 512, (g + 1) * 512)  # this core's 4 heads (features)
        wqT = tile_w((W_qkv[0:DIM] * SCALE).T[:, hs], 128)
        wkT = tile_w(W_qkv[DIM:2 * DIM].T[:, hs], 128)
        wvT = tile_w(W_qkv[2 * DIM:3 * DIM].T[:, hs], 512)[0]
        # w0T rows for this head group:
        # [512 din, 1024 cout] -> [8 ct, 128 p, 4 dc, 128 f]
        w0T = np.ascontiguousarray(
            W0.T[g * 512:(g + 1) * 512].reshape(4, 128, 8, 128)
            .transpose(2, 1, 0, 3)).astype(np.float16)
        bq = np.ascontiguousarray(
            (b_qkv[0:DIM] * SCALE)[hs].reshape(4, 128).T).astype(np.float32)
        xT = np.ascontiguousarray(
            x[b].T.reshape(8, 128, 2, 1024).transpose(2, 0, 1, 3)
        ).astype(np.float16)
        in_maps.append({
            "xT": xT, "wqT": wqT, "wkT": wkT, "wvT": wvT, "w0T": w0T,
            "bq": bq, "b0": b0r,
        })
    return in_maps


def _assemble(results):
    y = np.empty((B, N, DIM), dtype=np.float32)
    for b in range(B):
        y[b] = (results[2 * b]["out"].astype(np.float32)
                + results[2 * b + 1]["out"].astype(np.float32)).T
    return y


def kernel(x, W_qkv, b_qkv, W0, b0):
    nc = _get_nc()
    in_maps = _make_in_maps(x, W_qkv, b_qkv, W0, b0)
    res = run_bass_kernel_spmd(nc, in_maps, core_ids=list(range(NCORES)))
    return _assemble(res.results)


def kernel_traced(x, W_qkv, b_qkv, W0, b0, tmpdir=None):
    """Same as kernel() but with NTFF profiling; returns (output, BassKernelResults)."""
    nc = _get_nc()
    in_maps = _make_in_maps(x, W_qkv, b_qkv, W0, b0)
    res = run_bass_kernel_spmd(nc, in_maps, core_ids=list(range(NCORES)),
                               trace=True, trace_cores=[0], tmpdir=tmpdir)
    return _assemble(res.results), res
